# revision 13
# baseline (speedup 1.0000x reference)
"""Trainium2 Bass kernel for BlockAttnResLayer.

Computation (reference):
  V = concat([blocks, partial[None]])            # [9, B*T, D]
  rms = sqrt(mean(V^2, -1) + 1e-8)
  logits[n,t] = (V[n,t,:] . (norm_scale*proj_w)) / rms[n,t]
  alpha = softmax(logits, axis=n)
  h = sum_n alpha * V
  f = gelu(h @ W1) @ W2                          # tanh-approx gelu
  new_partial = partial + f
  returns (h, new_partial)

Sharding: pure data-parallel over tokens (B*T = 4096 -> 512/core on 8 cores),
weights replicated.

Numerics: everything streams as fp16 (V, W1, W2, h, activations, outputs);
matmuls accumulate fp32 in PSUM, softmax/rms math in fp32.  Measured
end-to-end error vs the fp64 reference is ~4e-4 (gate 2e-2).  fp16 matmuls
run 1 cycle/row on the PE with fast-weight-load (vs 236 ns LDWEIGHTS for
fp32r), fp16 DVE ops run in 2x packed mode, and DMA bytes halve vs fp32.

Structure: attention emits h (fp16, DMA'd out) and hT (PE-transposed, the
MM1 moving operand).  MM1: stationary W1 tiles [128d,128f], moving hT[k]
[128d,512tok], 8 f-chunks per PSUM group, gelu-evicted into a fully
resident fp16 act buffer [64][128f,512tok].  MM2: stationary act[fc] token
slices, moving W2 tiles [128f,512d]; 8 PSUM banks = 4 token tiles x 2
d-slices per quarter-F sweep, partials accumulated into an fp16 SBUF
accumulator so W1 and W2 are each read exactly once from HBM (~90 MB/core
total vs 271 MB for the fp32 baseline).
"""
import numpy as np
from contextlib import ExitStack

import concourse.bass as bass
import concourse.bacc as bacc
import concourse.tile as tile
from concourse import mybir
from concourse.bass_utils import run_bass_kernel_spmd
from concourse.masks import make_identity

f32 = mybir.dt.float32
f16 = mybir.dt.float16
AF = mybir.ActivationFunctionType
ALU = mybir.AluOpType

N_CORES = 8
NB = 8            # completed blocks
N1 = 9            # blocks + partial
B, T, D, F = 2, 2048, 2048, 8192
TOK = B * T       # 4096
TPC = TOK // N_CORES  # 512 tokens per core
P = 128
TT = TPC // P     # 4 token tiles per core
DC = D // P       # 16 d-chunks
FC = F // P       # 64 f-chunks
GF = 8            # f-chunks per MM1 PSUM group
NG1 = FC // GF    # 8 MM1 groups
NS2 = 4           # MM2 f-sweeps (16 f-chunks each)
SF = FC // NS2    # 16 f-chunks per MM2 sweep
EPS = 1e-8


def build_nc(n_reps: int = 1):
    nc = bacc.Bacc("TRN2", target_bir_lowering=False, debug=False, num_devices=N_CORES)
    vb = nc.dram_tensor("vb", [N1, TPC, D], f16, kind="ExternalInput").ap()
    # w1: [DC, P, F] with w1[k, p, f] = W1[k*128+p, f]  (plain reshape)
    w1 = nc.dram_tensor("w1", [DC, P, F], f16, kind="ExternalInput").ap()
    # w2: [FC, P, D] with w2[c, p, d] = W2[c*128+p, d]  (plain reshape)
    w2 = nc.dram_tensor("w2", [FC, P, D], f16, kind="ExternalInput").ap()
    # pw16 = norm_scale * proj_w, fp16 [D]
    pw = nc.dram_tensor("pw", [D], f16, kind="ExternalInput").ap()
    h_out = nc.dram_tensor("h_out", [TPC, D], f16, kind="ExternalOutput").ap()
    np_out = nc.dram_tensor("np_out", [TPC, D], f16, kind="ExternalOutput").ap()

    h_out_t = h_out.rearrange("(tt p) d -> tt p d", p=P)

    with tile.TileContext(nc) as tc, ExitStack() as ctx:
        consts = ctx.enter_context(tc.tile_pool(name="consts", bufs=1))
        ident = consts.tile([P, P], f16)
        make_identity(nc, ident)
        eps_t = consts.tile([P, 1], f32)
        nc.vector.memset(eps_t, EPS)
        pw_b = consts.tile([P, D], f16)
        pw_bcast = bass.AP(tensor=pw.tensor, offset=pw.offset, ap=[[0, P], *pw.ap])
        nc.gpsimd.dma_start(out=pw_b, in_=pw_bcast)

        # transposed h: hTs[k] is [128 d, 512 tok], fp16 (MM1 moving operand)
        hTp = ctx.enter_context(tc.tile_pool(name="hTp", bufs=1))
        hTs = [hTp.tile([P, TPC], f16, name=f"hT{k}") for k in range(DC)]
        # partial block kept resident for the residual add
        pbp = ctx.enter_context(tc.tile_pool(name="pbp", bufs=1))
        pbs = [pbp.tile([P, D], f16, name=f"pb{m}") for m in range(TT)]
        # weight stream pools (outer so prefetch runs during attention)
        w1p = ctx.enter_context(tc.tile_pool(name="w1p", bufs=10))   # [128,1024] f16
        w2p = ctx.enter_context(tc.tile_pool(name="w2p", bufs=10))   # [128,1024] f16

        for _rep in range(n_reps):
            # ---------------- Phase A: block attention -> h, hT ----------------
            # rms/dot stats on ACT/DVE/Pool; the weighted sum itself runs on
            # the (otherwise idle) PE as diag(e)-stationary matmuls that
            # accumulate un-normalized h in PSUM; the 1/sum(e) normalization
            # folds into the ACT eviction via a per-partition scale.
            with ExitStack() as ctxA:
                vpool = ctxA.enter_context(tc.tile_pool(name="vpool", bufs=8))
                scr = ctxA.enter_context(tc.tile_pool(name="scr", bufs=1))
                small = ctxA.enter_context(tc.tile_pool(name="small", bufs=3))
                dgp = ctxA.enter_context(tc.tile_pool(name="dgp", bufs=12))
                hpool = ctxA.enter_context(tc.tile_pool(name="hpool", bufs=2))
                psumT = ctxA.enter_context(tc.tile_pool(name="psumT", bufs=2,
                                                        space="PSUM"))
                psumH = ctxA.enter_context(tc.tile_pool(name="psumH", bufs=6,
                                                        space="PSUM"))

                sq_s = scr.tile([P, D], f16, name="sq_s")
                ds_s = scr.tile([P, D], f16, name="ds_s")
                ds_a = scr.tile([P, D], f16, name="ds_a")
                ds_b = scr.tile([P, D], f16, name="ds_b")

                ACT_DOTS = {7, 8}   # dot accumulations folded onto ACT

                for tt in range(TT):
                    ss9 = small.tile([P, N1], f32, name="ss9")
                    dp9 = small.tile([P, N1], f32, name="dp9")
                    vts = []
                    for n in range(N1):
                        v = pbs[tt] if n == NB else vpool.tile([P, D], f16, name="vt")
                        eng = nc.sync if n % 2 == 0 else nc.gpsimd
                        eng.dma_start(out=v, in_=vb[n, tt * P:(tt + 1) * P, :])
                        vts.append(v)
                        nc.scalar.activation(sq_s[:], v[:], AF.Square,
                                             accum_out=ss9[:, n:n + 1])
                        if n in ACT_DOTS:
                            # product on DVE (2x), accumulate on ACT (2x)
                            dsx = ds_a if n % 2 else ds_b
                            nc.vector.tensor_mul(dsx[:], v[:], pw_b[:])
                            nc.scalar.activation(dsx[:], dsx[:], AF.Copy,
                                                 accum_out=dp9[:, n:n + 1])
                        else:
                            # product on DVE (2x), accumulate via in-place
                            # tensor_scalar copy (4x) on DVE
                            nc.vector.tensor_mul(ds_s[:], v[:], pw_b[:])
                            nc.vector.tensor_scalar(
                                out=ds_s[:], in0=ds_s[:], scalar1=1.0,
                                scalar2=0.0, op0=ALU.mult, op1=ALU.add,
                                accum_out=dp9[:, n:n + 1])
                    rms9 = small.tile([P, N1], f32, name="rms9")
                    nc.scalar.activation(rms9[:], ss9[:], AF.Sqrt,
                                         bias=eps_t[:], scale=1.0 / D)
                    inv9 = small.tile([P, N1], f32, name="inv9")
                    nc.vector.reciprocal(inv9[:], rms9[:])
                    lg9 = small.tile([P, N1], f32, name="lg9")
                    nc.vector.tensor_mul(lg9[:], dp9[:], inv9[:])
                    mx1 = small.tile([P, 1], f32, name="mx1")
                    nc.vector.tensor_reduce(mx1[:], lg9[:], axis=mybir.AxisListType.X,
                                            op=ALU.max)
                    nc.vector.tensor_scalar_sub(lg9[:], lg9[:], mx1[:])
                    e9 = small.tile([P, N1], f32, name="e9")
                    se1 = small.tile([P, 1], f32, name="se1")
                    nc.scalar.activation(e9[:], lg9[:], AF.Exp, accum_out=se1[:])
                    invs = small.tile([P, 1], f32, name="invs")
                    nc.vector.reciprocal(invs[:], se1[:])

                    # diag(e_n) = identity * e_n (per-partition scalar)
                    dgs = []
                    for n in range(N1):
                        dg = dgp.tile([P, P], f16, name="dg")
                        nc.vector.tensor_scalar_mul(dg[:], ident[:],
                                                    e9[:, n:n + 1])
                        dgs.append(dg)
                    # h_unnorm[t, :] = sum_n e_n[t] * V_n[t, :]  on the PE
                    hps = [psumH.tile([P, 512], f32, name="hps") for _ in range(4)]
                    for n in range(N1):
                        for g in range(4):
                            nc.tensor.matmul(
                                hps[g][:], lhsT=dgs[n][:],
                                rhs=vts[n][:, g * 512:(g + 1) * 512],
                                start=(n == 0), stop=(n == N1 - 1))
                    h_t = hpool.tile([P, D], f16, name="ht")
                    for g in range(4):
                        nc.scalar.activation(h_t[:, g * 512:(g + 1) * 512],
                                             hps[g][:], AF.Copy, scale=invs[:])
                    nc.sync.dma_start(out=h_out_t[tt], in_=h_t[:])
                    for k in range(DC):
                        pst = psumT.tile([P, P], f16, name="pst")
                        nc.tensor.transpose(pst[:], h_t[:, k * P:(k + 1) * P], ident[:])
                        nc.scalar.activation(
                            hTs[k][:, tt * P:(tt + 1) * P], pst[:], AF.Copy)

            # ---------------- Phase B: FFN (fp16) + residual ----------------
            with ExitStack() as ctxB:
                actp = ctxB.enter_context(tc.tile_pool(name="actp", bufs=1))
                accp = ctxB.enter_context(tc.tile_pool(name="accp", bufs=1))
                evp = ctxB.enter_context(tc.tile_pool(name="evp", bufs=4))

                acts = [actp.tile([P, TPC], f16, name=f"act{c}") for c in range(FC)]

                # ---- MM1: act[fc] = gelu(hT^T @ W1[:, fc]) ----
                ctxB1 = ctxB.enter_context(ExitStack())
                psM1 = ctxB1.enter_context(tc.tile_pool(name="psM1", bufs=8,
                                                        space="PSUM"))
                for g in range(NG1):            # 8 groups of GF=8 f-chunks
                    ps1 = [psM1.tile([P, TPC], f32, name="ps") for _ in range(GF)]
                    for k in range(DC):
                        w1t = w1p.tile([P, GF * P], f16, name="w1t")
                        nc.scalar.dma_start(
                            out=w1t, in_=w1[k, :, g * GF * P:(g + 1) * GF * P])
                        for j in range(GF):
                            nc.tensor.matmul(
                                ps1[j][:], lhsT=w1t[:, j * P:(j + 1) * P],
                                rhs=hTs[k][:], start=(k == 0), stop=(k == DC - 1))
                    for j in range(GF):
                        nc.scalar.activation(acts[g * GF + j][:], ps1[j][:],
                                             AF.Gelu_apprx_tanh)

                ctxB1.close()

                # ---- MM2: np = act^T-slices @ W2 + partial ----
                ctxB2 = ctxB.enter_context(ExitStack())
                psM2 = ctxB2.enter_context(tc.tile_pool(name="psM2", bufs=8,
                                                        space="PSUM"))
                accs = [accp.tile([P, D], f16, name=f"acc{m}") for m in range(TT)]
                for dh in range(2):             # d halves (1024 wide)
                    for s in range(NS2):        # quarter-F sweeps
                        ps2 = [psM2.tile([P, 512], f32, name="ps2w")
                               for _ in range(8)]
                        for fl in range(SF):
                            fc = s * SF + fl
                            w2t = w2p.tile([P, 1024], f16, name="w2t")
                            nc.gpsimd.dma_start(
                                out=w2t, in_=w2[fc, :, dh * 1024:(dh + 1) * 1024])
                            for m in range(TT):
                                for qh in range(2):
                                    nc.tensor.matmul(
                                        ps2[m * 2 + qh][:],
                                        lhsT=acts[fc][:, m * P:(m + 1) * P],
                                        rhs=w2t[:, qh * 512:(qh + 1) * 512],
                                        start=(fl == 0), stop=(fl == SF - 1))
                        for m in range(TT):
                            for qh in range(2):
                                col = dh * 1024 + qh * 512
                                dst = accs[m][:, col:col + 512]
                                srcp = ps2[m * 2 + qh][:]
                                if s == 0:
                                    nc.vector.tensor_copy(dst, srcp)
                                elif s < NS2 - 1:
                                    nc.vector.tensor_add(dst, srcp, dst)
                                else:
                                    ev = evp.tile([P, 512], f16, name="ev")
                                    nc.vector.tensor_add(
                                        ev[:], srcp, pbs[m][:, col:col + 512])
                                    nc.vector.tensor_add(ev[:], ev[:], dst)
                                    nc.sync.dma_start(
                                        out=np_out[m * P:(m + 1) * P, col:col + 512],
                                        in_=ev[:])
                ctxB2.close()

    nc.compile()
    return nc


_NC = None


def _get_nc():
    global _NC
    if _NC is None:
        _NC = build_nc()
    return _NC


def prep_inputs(blocks, partial_block, proj_w, norm_scale, ffn_w1, ffn_w2):
    """Host-side shard + fp16 conversion; returns per-core input maps."""
    blocks = np.asarray(blocks, dtype=np.float32).reshape(NB, TOK, D)
    pb = np.asarray(partial_block, dtype=np.float32).reshape(TOK, D)
    w1r = np.asarray(ffn_w1, dtype=np.float16).reshape(DC, P, F)
    w2r = np.asarray(ffn_w2, dtype=np.float16).reshape(FC, P, D)
    pw16 = (np.asarray(norm_scale, np.float32)
            * np.asarray(proj_w, np.float32)).astype(np.float16)

    in_maps = []
    for c in range(N_CORES):
        sl = slice(c * TPC, (c + 1) * TPC)
        vbc = np.concatenate([blocks[:, sl], pb[None, sl]],
                             axis=0).astype(np.float16)
        in_maps.append({"vb": vbc, "w1": w1r, "w2": w2r, "pw": pw16})
    return in_maps


def kernel(blocks, partial_block, proj_w, norm_scale, ffn_w1, ffn_w2):
    in_maps = prep_inputs(blocks, partial_block, proj_w, norm_scale,
                          ffn_w1, ffn_w2)
    nc = _get_nc()
    res = run_bass_kernel_spmd(nc, in_maps, list(range(N_CORES)))
    h = np.concatenate([r["h_out"] for r in res.results],
                       axis=0).astype(np.float32).reshape(B, T, D)
    npar = np.concatenate([r["np_out"] for r in res.results],
                          axis=0).astype(np.float32).reshape(B, T, D)
    return h, npar


# revision 14
# speedup vs baseline: 1.2193x; 1.2193x over previous
"""Trainium2 Bass kernel for BlockAttnResLayer.

Computation (reference):
  V = concat([blocks, partial[None]])            # [9, B*T, D]
  rms = sqrt(mean(V^2, -1) + 1e-8)
  logits[n,t] = (V[n,t,:] . (norm_scale*proj_w)) / rms[n,t]
  alpha = softmax(logits, axis=n)
  h = sum_n alpha * V
  f = gelu(h @ W1) @ W2                          # tanh-approx gelu
  new_partial = partial + f
  returns (h, new_partial)

Sharding: pure data-parallel over tokens (B*T = 4096 -> 512/core on 8 cores),
weights replicated.

Numerics: everything streams as fp16 (V, W1, W2, h, activations, outputs);
matmuls accumulate fp32 in PSUM, softmax/rms math in fp32.  Measured
end-to-end error vs the fp64 reference is ~4e-4 (gate 2e-2).  fp16 matmuls
run 1 cycle/row on the PE with fast-weight-load (vs 236 ns LDWEIGHTS for
fp32r), fp16 DVE ops run in 2x packed mode, and DMA bytes halve vs fp32.

Structure: attention emits h (fp16, DMA'd out) and hT (PE-transposed, the
MM1 moving operand).  MM1: stationary W1 tiles [128d,128f], moving hT[k]
[128d,512tok], 8 f-chunks per PSUM group, gelu-evicted into a fully
resident fp16 act buffer [64][128f,512tok].  MM2: stationary act[fc] token
slices, moving W2 tiles [128f,512d]; 8 PSUM banks = 4 token tiles x 2
d-slices per quarter-F sweep, partials accumulated into an fp16 SBUF
accumulator so W1 and W2 are each read exactly once from HBM (~90 MB/core
total vs 271 MB for the fp32 baseline).
"""
import numpy as np
from contextlib import ExitStack

import concourse.bass as bass
import concourse.bacc as bacc
import concourse.tile as tile
from concourse import mybir
from concourse.bass_utils import run_bass_kernel_spmd
from concourse.masks import make_identity

f32 = mybir.dt.float32
f16 = mybir.dt.float16
AF = mybir.ActivationFunctionType
ALU = mybir.AluOpType

N_CORES = 8
NB = 8            # completed blocks
N1 = 9            # blocks + partial
B, T, D, F = 2, 2048, 2048, 8192
TOK = B * T       # 4096
TPC = TOK // N_CORES  # 512 tokens per core
P = 128
TT = TPC // P     # 4 token tiles per core
DC = D // P       # 16 d-chunks
FC = F // P       # 64 f-chunks
GF = 8            # f-chunks per MM1 PSUM group
NG1 = FC // GF    # 8 MM1 groups
NS2 = 4           # MM2 f-sweeps (16 f-chunks each)
SF = FC // NS2    # 16 f-chunks per MM2 sweep
EPS = 1e-8


def build_nc(n_reps: int = 1):
    nc = bacc.Bacc("TRN2", target_bir_lowering=False, debug=False, num_devices=N_CORES)
    vb = nc.dram_tensor("vb", [N1, TPC, D], f16, kind="ExternalInput").ap()
    # w1: [DC, P, F] with w1[k, p, f] = W1[k*128+p, f]  (plain reshape)
    w1 = nc.dram_tensor("w1", [DC, P, F], f16, kind="ExternalInput").ap()
    # w2: [FC, P, D] with w2[c, p, d] = W2[c*128+p, d]  (plain reshape)
    w2 = nc.dram_tensor("w2", [FC, P, D], f16, kind="ExternalInput").ap()
    # pw16 = norm_scale * proj_w, fp16 [D]
    pw = nc.dram_tensor("pw", [D], f16, kind="ExternalInput").ap()
    h_out = nc.dram_tensor("h_out", [TPC, D], f16, kind="ExternalOutput").ap()
    np_out = nc.dram_tensor("np_out", [TPC, D], f16, kind="ExternalOutput").ap()

    h_out_t = h_out.rearrange("(tt p) d -> tt p d", p=P)

    with tile.TileContext(nc) as tc, ExitStack() as ctx:
        consts = ctx.enter_context(tc.tile_pool(name="consts", bufs=1))
        ident = consts.tile([P, P], f16)
        make_identity(nc, ident)
        eps_t = consts.tile([P, 1], f32)
        nc.vector.memset(eps_t, EPS)
        pw_b = consts.tile([P, D], f16)
        pw_bcast = bass.AP(tensor=pw.tensor, offset=pw.offset, ap=[[0, P], *pw.ap])
        nc.gpsimd.dma_start(out=pw_b, in_=pw_bcast)

        # transposed h: hTs[k] is [128 d, 512 tok], fp16 (MM1 moving operand)
        hTp = ctx.enter_context(tc.tile_pool(name="hTp", bufs=1))
        hTs = [hTp.tile([P, TPC], f16, name=f"hT{k}") for k in range(DC)]
        # partial block kept resident for the residual add
        pbp = ctx.enter_context(tc.tile_pool(name="pbp", bufs=1))
        pbs = [pbp.tile([P, D], f16, name=f"pb{m}") for m in range(TT)]
        # weight stream pools (outer so prefetch runs during attention)
        w1p = ctx.enter_context(tc.tile_pool(name="w1p", bufs=16))   # [128,1024] f16
        w2p = ctx.enter_context(tc.tile_pool(name="w2p", bufs=12))   # [128,1024] f16

        for _rep in range(n_reps):
            # ---------------- Phase A: block attention -> h, hT ----------------
            # rms/dot stats on ACT/DVE/Pool; the weighted sum itself runs on
            # the (otherwise idle) PE as diag(e)-stationary matmuls that
            # accumulate un-normalized h in PSUM; the 1/sum(e) normalization
            # folds into the ACT eviction via a per-partition scale.
            with ExitStack() as ctxA:
                vpool = ctxA.enter_context(tc.tile_pool(name="vpool", bufs=14))
                scr = ctxA.enter_context(tc.tile_pool(name="scr", bufs=1))
                small = ctxA.enter_context(tc.tile_pool(name="small", bufs=3))
                dgp = ctxA.enter_context(tc.tile_pool(name="dgp", bufs=12))
                hpool = ctxA.enter_context(tc.tile_pool(name="hpool", bufs=2))
                psumT = ctxA.enter_context(tc.tile_pool(name="psumT", bufs=2,
                                                        space="PSUM"))
                psumH = ctxA.enter_context(tc.tile_pool(name="psumH", bufs=6,
                                                        space="PSUM"))

                sq_s = scr.tile([P, D], f16, name="sq_s")
                ds_s = scr.tile([P, D], f16, name="ds_s")
                ds_a = scr.tile([P, D], f16, name="ds_a")
                ds_b = scr.tile([P, D], f16, name="ds_b")

                ACT_DOTS = {0, 2, 4, 6, 8}   # dot accumulations folded onto ACT

                for tt in range(TT):
                    ss9 = small.tile([P, N1], f32, name="ss9")
                    dp9 = small.tile([P, N1], f32, name="dp9")
                    vts = []
                    for n in range(N1):
                        v = pbs[tt] if n == NB else vpool.tile([P, D], f16, name="vt")
                        eng = (nc.sync, nc.gpsimd, nc.scalar)[n % 3]
                        eng.dma_start(out=v, in_=vb[n, tt * P:(tt + 1) * P, :])
                        vts.append(v)
                        nc.scalar.activation(sq_s[:], v[:], AF.Square,
                                             accum_out=ss9[:, n:n + 1])
                        if n in ACT_DOTS:
                            # product on DVE (2x), accumulate on ACT (2x)
                            dsx = ds_a if (n // 2) % 2 else ds_b
                            nc.vector.tensor_mul(dsx[:], v[:], pw_b[:])
                            nc.scalar.activation(dsx[:], dsx[:], AF.Copy,
                                                 accum_out=dp9[:, n:n + 1])
                        else:
                            # fused product+accumulate on DVE (1x STT)
                            nc.vector.scalar_tensor_tensor(
                                out=ds_s[:], in0=v[:], scalar=1.0, in1=pw_b[:],
                                op0=ALU.mult, op1=ALU.mult,
                                accum_out=dp9[:, n:n + 1])
                    rms9 = small.tile([P, N1], f32, name="rms9")
                    nc.scalar.activation(rms9[:], ss9[:], AF.Sqrt,
                                         bias=eps_t[:], scale=1.0 / D)
                    inv9 = small.tile([P, N1], f32, name="inv9")
                    nc.vector.reciprocal(inv9[:], rms9[:])
                    lg9 = small.tile([P, N1], f32, name="lg9")
                    nc.vector.tensor_mul(lg9[:], dp9[:], inv9[:])
                    e9 = small.tile([P, N1], f32, name="e9")
                    se1 = small.tile([P, 1], f32, name="se1")
                    nc.scalar.activation(e9[:], lg9[:], AF.Exp, accum_out=se1[:])
                    invs = small.tile([P, 1], f32, name="invs")
                    nc.vector.reciprocal(invs[:], se1[:])

                    # diag(e_n) = identity * e_n (per-partition scalar)
                    dgs = []
                    for n in range(N1):
                        dg = dgp.tile([P, P], f16, name="dg")
                        nc.vector.tensor_scalar_mul(dg[:], ident[:],
                                                    e9[:, n:n + 1])
                        dgs.append(dg)
                    # h_unnorm[t, :] = sum_n e_n[t] * V_n[t, :]  on the PE
                    hps = [psumH.tile([P, 512], f32, name="hps") for _ in range(4)]
                    for n in range(N1):
                        for g in range(4):
                            nc.tensor.matmul(
                                hps[g][:], lhsT=dgs[n][:],
                                rhs=vts[n][:, g * 512:(g + 1) * 512],
                                start=(n == 0), stop=(n == N1 - 1))
                    h_t = hpool.tile([P, D], f16, name="ht")
                    for g in range(4):
                        nc.scalar.activation(h_t[:, g * 512:(g + 1) * 512],
                                             hps[g][:], AF.Copy, scale=invs[:])
                    nc.sync.dma_start(out=h_out_t[tt], in_=h_t[:])
                    for k in range(DC):
                        pst = psumT.tile([P, P], f16, name="pst")
                        nc.tensor.transpose(pst[:], h_t[:, k * P:(k + 1) * P], ident[:])
                        nc.vector.tensor_copy(
                            hTs[k][:, tt * P:(tt + 1) * P], pst[:])

            # ---------------- Phase B: FFN (fp16) + residual ----------------
            with ExitStack() as ctxB:
                actp = ctxB.enter_context(tc.tile_pool(name="actp", bufs=1))
                accp = ctxB.enter_context(tc.tile_pool(name="accp", bufs=1))
                evp = ctxB.enter_context(tc.tile_pool(name="evp", bufs=4))

                acts = [actp.tile([P, TPC], f16, name=f"act{c}") for c in range(FC)]

                # ---- MM1: act[fc] = gelu(hT^T @ W1[:, fc]) ----
                ctxB1 = ctxB.enter_context(ExitStack())
                psM1 = ctxB1.enter_context(tc.tile_pool(name="psM1", bufs=8,
                                                        space="PSUM"))
                for g in range(NG1):            # 8 groups of GF=8 f-chunks
                    ps1 = [psM1.tile([P, TPC], f32, name="ps") for _ in range(GF)]
                    for k in range(DC):
                        w1t = w1p.tile([P, GF * P], f16, name="w1t")
                        eng = (nc.scalar, nc.sync, nc.gpsimd)[(g * DC + k) % 3]
                        eng.dma_start(
                            out=w1t, in_=w1[k, :, g * GF * P:(g + 1) * GF * P])
                        for j in range(GF):
                            nc.tensor.matmul(
                                ps1[j][:], lhsT=w1t[:, j * P:(j + 1) * P],
                                rhs=hTs[k][:], start=(k == 0), stop=(k == DC - 1))
                    for j in range(GF):
                        nc.scalar.activation(acts[g * GF + j][:], ps1[j][:],
                                             AF.Gelu_apprx_tanh)

                ctxB1.close()

                # ---- MM2: np = act^T-slices @ W2 + partial ----
                ctxB2 = ctxB.enter_context(ExitStack())
                psM2 = ctxB2.enter_context(tc.tile_pool(name="psM2", bufs=8,
                                                        space="PSUM"))
                accs = [accp.tile([P, D], f16, name=f"acc{m}") for m in range(TT)]
                for dh in range(2):             # d halves (1024 wide)
                    for s in range(NS2):        # quarter-F sweeps
                        ps2 = [psM2.tile([P, 512], f32, name="ps2w")
                               for _ in range(8)]
                        for fl in range(SF):
                            fc = s * SF + fl
                            w2t = w2p.tile([P, 1024], f16, name="w2t")
                            eng = (nc.gpsimd, nc.scalar, nc.sync)[(dh * FC + fc) % 3]
                            eng.dma_start(
                                out=w2t, in_=w2[fc, :, dh * 1024:(dh + 1) * 1024])
                            for m in range(TT):
                                for qh in range(2):
                                    nc.tensor.matmul(
                                        ps2[m * 2 + qh][:],
                                        lhsT=acts[fc][:, m * P:(m + 1) * P],
                                        rhs=w2t[:, qh * 512:(qh + 1) * 512],
                                        start=(fl == 0), stop=(fl == SF - 1))
                        for m in range(TT):
                            for qh in range(2):
                                col = dh * 1024 + qh * 512
                                dst = accs[m][:, col:col + 512]
                                srcp = ps2[m * 2 + qh][:]
                                if s == 0:
                                    nc.vector.tensor_copy(dst, srcp)
                                elif s < NS2 - 1:
                                    nc.vector.tensor_add(dst, srcp, dst)
                                else:
                                    ev = evp.tile([P, 512], f16, name="ev")
                                    nc.vector.tensor_add(
                                        ev[:], srcp, pbs[m][:, col:col + 512])
                                    nc.vector.tensor_add(ev[:], ev[:], dst)
                                    nc.sync.dma_start(
                                        out=np_out[m * P:(m + 1) * P, col:col + 512],
                                        in_=ev[:])
                ctxB2.close()

    nc.compile()
    return nc


_NC = None


def _get_nc():
    global _NC
    if _NC is None:
        _NC = build_nc()
    return _NC


def prep_inputs(blocks, partial_block, proj_w, norm_scale, ffn_w1, ffn_w2):
    """Host-side shard + fp16 conversion; returns per-core input maps."""
    blocks = np.asarray(blocks, dtype=np.float32).reshape(NB, TOK, D)
    pb = np.asarray(partial_block, dtype=np.float32).reshape(TOK, D)
    w1r = np.asarray(ffn_w1, dtype=np.float16).reshape(DC, P, F)
    w2r = np.asarray(ffn_w2, dtype=np.float16).reshape(FC, P, D)
    pw16 = (np.asarray(norm_scale, np.float32)
            * np.asarray(proj_w, np.float32)).astype(np.float16)

    in_maps = []
    for c in range(N_CORES):
        sl = slice(c * TPC, (c + 1) * TPC)
        vbc = np.concatenate([blocks[:, sl], pb[None, sl]],
                             axis=0).astype(np.float16)
        in_maps.append({"vb": vbc, "w1": w1r, "w2": w2r, "pw": pw16})
    return in_maps


def kernel(blocks, partial_block, proj_w, norm_scale, ffn_w1, ffn_w2):
    in_maps = prep_inputs(blocks, partial_block, proj_w, norm_scale,
                          ffn_w1, ffn_w2)
    nc = _get_nc()
    res = run_bass_kernel_spmd(nc, in_maps, list(range(N_CORES)))
    h = np.concatenate([r["h_out"] for r in res.results],
                       axis=0).astype(np.float32).reshape(B, T, D)
    npar = np.concatenate([r["np_out"] for r in res.results],
                          axis=0).astype(np.float32).reshape(B, T, D)
    return h, npar


# revision 15
# speedup vs baseline: 1.2248x; 1.0045x over previous
"""Trainium2 Bass kernel for BlockAttnResLayer.

Computation (reference):
  V = concat([blocks, partial[None]])            # [9, B*T, D]
  rms = sqrt(mean(V^2, -1) + 1e-8)
  logits[n,t] = (V[n,t,:] . (norm_scale*proj_w)) / rms[n,t]
  alpha = softmax(logits, axis=n)
  h = sum_n alpha * V
  f = gelu(h @ W1) @ W2                          # tanh-approx gelu
  new_partial = partial + f
  returns (h, new_partial)

Sharding: pure data-parallel over tokens (B*T = 4096 -> 512/core on 8 cores),
weights replicated.

Numerics: everything streams as fp16 (V, W1, W2, h, activations, outputs);
matmuls accumulate fp32 in PSUM, softmax/rms math in fp32.  Measured
end-to-end error vs the fp64 reference is ~4e-4 (gate 2e-2).  fp16 matmuls
run 1 cycle/row on the PE with fast-weight-load (vs 236 ns LDWEIGHTS for
fp32r), fp16 DVE ops run in 2x packed mode, and DMA bytes halve vs fp32.

Structure: attention emits h (fp16, DMA'd out) and hT (PE-transposed, the
MM1 moving operand).  MM1: stationary W1 tiles [128d,128f], moving hT[k]
[128d,512tok], 8 f-chunks per PSUM group, gelu-evicted into a fully
resident fp16 act buffer [64][128f,512tok].  MM2: stationary act[fc] token
slices, moving W2 tiles [128f,512d]; 8 PSUM banks = 4 token tiles x 2
d-slices per quarter-F sweep, partials accumulated into an fp16 SBUF
accumulator so W1 and W2 are each read exactly once from HBM (~90 MB/core
total vs 271 MB for the fp32 baseline).
"""
import numpy as np
from contextlib import ExitStack

import concourse.bass as bass
import concourse.bacc as bacc
import concourse.tile as tile
from concourse import mybir
from concourse.bass_utils import run_bass_kernel_spmd
from concourse.masks import make_identity

f32 = mybir.dt.float32
f16 = mybir.dt.float16
AF = mybir.ActivationFunctionType
ALU = mybir.AluOpType

N_CORES = 8
NB = 8            # completed blocks
N1 = 9            # blocks + partial
B, T, D, F = 2, 2048, 2048, 8192
TOK = B * T       # 4096
TPC = TOK // N_CORES  # 512 tokens per core
P = 128
TT = TPC // P     # 4 token tiles per core
DC = D // P       # 16 d-chunks
FC = F // P       # 64 f-chunks
GF = 8            # f-chunks per MM1 PSUM group
NG1 = FC // GF    # 8 MM1 groups
NS2 = 4           # MM2 f-sweeps (16 f-chunks each)
SF = FC // NS2    # 16 f-chunks per MM2 sweep
EPS = 1e-8


def build_nc(n_reps: int = 1):
    nc = bacc.Bacc("TRN2", target_bir_lowering=False, debug=False, num_devices=N_CORES)
    vb = nc.dram_tensor("vb", [N1, TPC, D], f16, kind="ExternalInput").ap()
    # w1: [DC, P, F] with w1[k, p, f] = W1[k*128+p, f]  (plain reshape)
    w1 = nc.dram_tensor("w1", [DC, P, F], f16, kind="ExternalInput").ap()
    # w2: [FC, P, D] with w2[c, p, d] = W2[c*128+p, d]  (plain reshape)
    w2 = nc.dram_tensor("w2", [FC, P, D], f16, kind="ExternalInput").ap()
    # pw16 = norm_scale * proj_w, fp16 [D]
    pw = nc.dram_tensor("pw", [D], f16, kind="ExternalInput").ap()
    h_out = nc.dram_tensor("h_out", [TPC, D], f16, kind="ExternalOutput").ap()
    np_out = nc.dram_tensor("np_out", [TPC, D], f16, kind="ExternalOutput").ap()

    h_out_t = h_out.rearrange("(tt p) d -> tt p d", p=P)

    with tile.TileContext(nc) as tc, ExitStack() as ctx:
        consts = ctx.enter_context(tc.tile_pool(name="consts", bufs=1))
        ident = consts.tile([P, P], f16)
        make_identity(nc, ident)
        eps_t = consts.tile([P, 1], f32)
        nc.vector.memset(eps_t, EPS)
        pw_b = consts.tile([P, D], f16)
        pw_bcast = bass.AP(tensor=pw.tensor, offset=pw.offset, ap=[[0, P], *pw.ap])
        nc.gpsimd.dma_start(out=pw_b, in_=pw_bcast)

        # transposed h: hTs[k] is [128 d, 512 tok], fp16 (MM1 moving operand)
        hTp = ctx.enter_context(tc.tile_pool(name="hTp", bufs=1))
        hTs = [hTp.tile([P, TPC], f16, name=f"hT{k}") for k in range(DC)]
        # partial block kept resident for the residual add
        pbp = ctx.enter_context(tc.tile_pool(name="pbp", bufs=1))
        pbs = [pbp.tile([P, D], f16, name=f"pb{m}") for m in range(TT)]
        # weight stream pools (outer so prefetch runs during attention)
        w1p = ctx.enter_context(tc.tile_pool(name="w1p", bufs=16))   # [128,1024] f16
        w2p = ctx.enter_context(tc.tile_pool(name="w2p", bufs=12))   # [128,1024] f16

        for _rep in range(n_reps):
            # ---------------- Phase A: block attention -> h, hT ----------------
            # rms/dot stats on ACT/DVE/Pool; the weighted sum itself runs on
            # the (otherwise idle) PE as diag(e)-stationary matmuls that
            # accumulate un-normalized h in PSUM; the 1/sum(e) normalization
            # folds into the ACT eviction via a per-partition scale.
            with ExitStack() as ctxA:
                vpool = ctxA.enter_context(tc.tile_pool(name="vpool", bufs=14))
                scr = ctxA.enter_context(tc.tile_pool(name="scr", bufs=1))
                small = ctxA.enter_context(tc.tile_pool(name="small", bufs=3))
                dgp = ctxA.enter_context(tc.tile_pool(name="dgp", bufs=12))
                hpool = ctxA.enter_context(tc.tile_pool(name="hpool", bufs=2))
                psumT = ctxA.enter_context(tc.tile_pool(name="psumT", bufs=2,
                                                        space="PSUM"))
                psumH = ctxA.enter_context(tc.tile_pool(name="psumH", bufs=6,
                                                        space="PSUM"))

                sq_s = scr.tile([P, D], f16, name="sq_s")
                ds_s = scr.tile([P, D], f16, name="ds_s")
                ds_a = scr.tile([P, D], f16, name="ds_a")
                ds_b = scr.tile([P, D], f16, name="ds_b")

                ACT_DOTS = {3, 7}   # dot accumulations folded onto ACT

                for tt in range(TT):
                    ss9 = small.tile([P, N1], f32, name="ss9")
                    dp9 = small.tile([P, N1], f32, name="dp9")
                    vts = []
                    for n in range(N1):
                        v = pbs[tt] if n == NB else vpool.tile([P, D], f16, name="vt")
                        eng = (nc.sync, nc.gpsimd)[n % 2]
                        eng.dma_start(out=v, in_=vb[n, tt * P:(tt + 1) * P, :])
                        vts.append(v)
                        nc.scalar.activation(sq_s[:], v[:], AF.Square,
                                             accum_out=ss9[:, n:n + 1])
                        if n in ACT_DOTS:
                            # product on DVE (2x), accumulate on ACT (2x)
                            dsx = ds_a if n == 3 else ds_b
                            nc.vector.tensor_mul(dsx[:], v[:], pw_b[:])
                            nc.scalar.activation(dsx[:], dsx[:], AF.Copy,
                                                 accum_out=dp9[:, n:n + 1])
                        else:
                            # fused product+accumulate on DVE (1x STT)
                            nc.vector.scalar_tensor_tensor(
                                out=ds_s[:], in0=v[:], scalar=1.0, in1=pw_b[:],
                                op0=ALU.mult, op1=ALU.mult,
                                accum_out=dp9[:, n:n + 1])
                    rms9 = small.tile([P, N1], f32, name="rms9")
                    nc.scalar.activation(rms9[:], ss9[:], AF.Sqrt,
                                         bias=eps_t[:], scale=1.0 / D)
                    inv9 = small.tile([P, N1], f32, name="inv9")
                    nc.vector.reciprocal(inv9[:], rms9[:])
                    lg9 = small.tile([P, N1], f32, name="lg9")
                    nc.vector.tensor_mul(lg9[:], dp9[:], inv9[:])
                    e9 = small.tile([P, N1], f32, name="e9")
                    se1 = small.tile([P, 1], f32, name="se1")
                    nc.scalar.activation(e9[:], lg9[:], AF.Exp, accum_out=se1[:])
                    invs = small.tile([P, 1], f32, name="invs")
                    nc.vector.reciprocal(invs[:], se1[:])

                    # diag(e_n) = identity * e_n (per-partition scalar)
                    dgs = []
                    for n in range(N1):
                        dg = dgp.tile([P, P], f16, name="dg")
                        nc.vector.tensor_scalar_mul(dg[:], ident[:],
                                                    e9[:, n:n + 1])
                        dgs.append(dg)
                    # h_unnorm[t, :] = sum_n e_n[t] * V_n[t, :]  on the PE
                    hps = [psumH.tile([P, 512], f32, name="hps") for _ in range(4)]
                    for n in range(N1):
                        for g in range(4):
                            nc.tensor.matmul(
                                hps[g][:], lhsT=dgs[n][:],
                                rhs=vts[n][:, g * 512:(g + 1) * 512],
                                start=(n == 0), stop=(n == N1 - 1))
                    h_t = hpool.tile([P, D], f16, name="ht")
                    for g in range(4):
                        nc.scalar.activation(h_t[:, g * 512:(g + 1) * 512],
                                             hps[g][:], AF.Copy, scale=invs[:])
                    nc.sync.dma_start(out=h_out_t[tt], in_=h_t[:])
                    for k in range(DC):
                        pst = psumT.tile([P, P], f16, name="pst")
                        nc.tensor.transpose(pst[:], h_t[:, k * P:(k + 1) * P], ident[:])
                        nc.scalar.activation(
                            hTs[k][:, tt * P:(tt + 1) * P], pst[:], AF.Copy)

            # ---------------- Phase B: FFN (fp16) + residual ----------------
            with ExitStack() as ctxB:
                actp = ctxB.enter_context(tc.tile_pool(name="actp", bufs=1))
                accp = ctxB.enter_context(tc.tile_pool(name="accp", bufs=1))
                evp = ctxB.enter_context(tc.tile_pool(name="evp", bufs=4))

                acts = [actp.tile([P, TPC], f16, name=f"act{c}") for c in range(FC)]

                # ---- MM1: act[fc] = gelu(hT^T @ W1[:, fc]) ----
                ctxB1 = ctxB.enter_context(ExitStack())
                psM1 = ctxB1.enter_context(tc.tile_pool(name="psM1", bufs=8,
                                                        space="PSUM"))
                for g in range(NG1):            # 8 groups of GF=8 f-chunks
                    ps1 = [psM1.tile([P, TPC], f32, name="ps") for _ in range(GF)]
                    for k in range(DC):
                        w1t = w1p.tile([P, GF * P], f16, name="w1t")
                        eng = (nc.sync, nc.gpsimd)[(g * DC + k) % 2]
                        eng.dma_start(
                            out=w1t, in_=w1[k, :, g * GF * P:(g + 1) * GF * P])
                        for j in range(GF):
                            nc.tensor.matmul(
                                ps1[j][:], lhsT=w1t[:, j * P:(j + 1) * P],
                                rhs=hTs[k][:], start=(k == 0), stop=(k == DC - 1))
                    for j in range(GF):
                        nc.scalar.activation(acts[g * GF + j][:], ps1[j][:],
                                             AF.Gelu_apprx_tanh)

                ctxB1.close()

                # ---- MM2: np = act^T-slices @ W2 + partial ----
                ctxB2 = ctxB.enter_context(ExitStack())
                psM2 = ctxB2.enter_context(tc.tile_pool(name="psM2", bufs=8,
                                                        space="PSUM"))
                accs = [accp.tile([P, D], f16, name=f"acc{m}") for m in range(TT)]
                for dh in range(2):             # d halves (1024 wide)
                    for s in range(NS2):        # quarter-F sweeps
                        ps2 = [psM2.tile([P, 512], f32, name="ps2w")
                               for _ in range(8)]
                        for fl in range(SF):
                            fc = s * SF + fl
                            w2t = w2p.tile([P, 1024], f16, name="w2t")
                            eng = (nc.gpsimd, nc.sync)[(dh * FC + fc) % 2]
                            eng.dma_start(
                                out=w2t, in_=w2[fc, :, dh * 1024:(dh + 1) * 1024])
                            for m in range(TT):
                                for qh in range(2):
                                    nc.tensor.matmul(
                                        ps2[m * 2 + qh][:],
                                        lhsT=acts[fc][:, m * P:(m + 1) * P],
                                        rhs=w2t[:, qh * 512:(qh + 1) * 512],
                                        start=(fl == 0), stop=(fl == SF - 1))
                        for m in range(TT):
                            for qh in range(2):
                                col = dh * 1024 + qh * 512
                                dst = accs[m][:, col:col + 512]
                                srcp = ps2[m * 2 + qh][:]
                                if s == 0:
                                    nc.vector.tensor_add(
                                        dst, srcp, pbs[m][:, col:col + 512])
                                elif s < NS2 - 1:
                                    nc.vector.tensor_add(dst, srcp, dst)
                                else:
                                    ev = evp.tile([P, 512], f16, name="ev")
                                    nc.vector.tensor_add(ev[:], srcp, dst)
                                    nc.sync.dma_start(
                                        out=np_out[m * P:(m + 1) * P, col:col + 512],
                                        in_=ev[:])
                ctxB2.close()

    nc.compile()
    return nc


_NC = None


def _get_nc():
    global _NC
    if _NC is None:
        _NC = build_nc()
    return _NC


def prep_inputs(blocks, partial_block, proj_w, norm_scale, ffn_w1, ffn_w2):
    """Host-side shard + fp16 conversion; returns per-core input maps."""
    blocks = np.asarray(blocks, dtype=np.float32).reshape(NB, TOK, D)
    pb = np.asarray(partial_block, dtype=np.float32).reshape(TOK, D)
    w1r = np.asarray(ffn_w1, dtype=np.float16).reshape(DC, P, F)
    w2r = np.asarray(ffn_w2, dtype=np.float16).reshape(FC, P, D)
    pw16 = (np.asarray(norm_scale, np.float32)
            * np.asarray(proj_w, np.float32)).astype(np.float16)

    in_maps = []
    for c in range(N_CORES):
        sl = slice(c * TPC, (c + 1) * TPC)
        vbc = np.concatenate([blocks[:, sl], pb[None, sl]],
                             axis=0).astype(np.float16)
        in_maps.append({"vb": vbc, "w1": w1r, "w2": w2r, "pw": pw16})
    return in_maps


def kernel(blocks, partial_block, proj_w, norm_scale, ffn_w1, ffn_w2):
    in_maps = prep_inputs(blocks, partial_block, proj_w, norm_scale,
                          ffn_w1, ffn_w2)
    nc = _get_nc()
    res = run_bass_kernel_spmd(nc, in_maps, list(range(N_CORES)))
    h = np.concatenate([r["h_out"] for r in res.results],
                       axis=0).astype(np.float32).reshape(B, T, D)
    npar = np.concatenate([r["np_out"] for r in res.results],
                          axis=0).astype(np.float32).reshape(B, T, D)
    return h, npar
